# revision 5
# baseline (speedup 1.0000x reference)
"""Multi-head attention (B=4, S=2048, D=1024, H=16, d_k=64) on 8 NeuronCores.

Sharding: core c handles batch b=c//2 and head-group g=c%2 (8 heads, 512
features). Each core computes Q/K/V projections for its head group,
attention, and a partial output projection (row-split W_o). Host sums the
two partial outputs per batch.

v2: software-pipelined emission. The attention phase is ACT(exp)-bound, so
all projection and output-projection matmuls that are not needed up front
are emitted to fill the Tensor-engine gaps inside the attention stream:
- prefix: kt[0] (DMA-paced), kt[1..3] as filler, V for head-pair 0,
  qt[0][sg=0] -> attention starts ~30us in.
- steady state: per (sg, hp) attention block; qt/V projections for later
  blocks and the per-sg output projection interleave into PE gaps.
- per-(fc,sg) split tiles for qt/kt/ctx and per-(t,hp) V tiles keep
  producer/consumer dependencies tile-granular (no false serialization).

Numerics identical to v1 (all matmuls bf16, fp32 PSUM, exp without
max-subtraction is safe: |score/8| <= ~7).
"""

import sys

sys.path.insert(0, "/opt/trn_rl_repo")

import numpy as np
import ml_dtypes

BF = ml_dtypes.bfloat16

S = 2048          # sequence length
D = 1024          # model dim
F = 512           # features per core (8 heads x 64)
HPC = 8           # heads per core
DK = 64           # head dim
P = 128           # partitions
NCORES = 8
KC = D // P       # 8 contraction chunks for projections
ST = S // P       # 16 sequence tiles of 128
SG = S // 512     # 4 sequence groups of 512
FC = F // P       # 4 feature chunks of 128 (= head pairs)


def _build_program(reps=1):
    import concourse.bass as bass
    import concourse.mybir as mybir
    import concourse.tile as tile
    from concourse import bacc

    dt = mybir.dt
    f32 = dt.float32
    bf16 = dt.bfloat16
    EXP = mybir.ActivationFunctionType.Exp

    nc = bacc.Bacc("TRN2", target_bir_lowering=False, debug=False,
                   num_devices=NCORES)

    xq_d = nc.declare_dram_parameter("xq", [D, S], bf16, isOutput=False)
    xk_d = nc.declare_dram_parameter("xk", [D, S], bf16, isOutput=False)
    xv_d = nc.declare_dram_parameter("xv", [D, S], bf16, isOutput=False)
    wq_d = nc.declare_dram_parameter("wq", [D, F], bf16, isOutput=False)
    wk_d = nc.declare_dram_parameter("wk", [D, F], bf16, isOutput=False)
    wv_d = nc.declare_dram_parameter("wv", [D, F], bf16, isOutput=False)
    wo_d = nc.declare_dram_parameter("wo", [F, D], bf16, isOutput=False)
    out_d = nc.declare_dram_parameter("out", [S, D], f32, isOutput=True)

    xq_t = xq_d.ap().rearrange("(c p) s -> c p s", p=P)
    xk_t = xk_d.ap().rearrange("(c p) s -> c p s", p=P)
    xv_t = xv_d.ap().rearrange("(c p) s -> c p s", p=P)
    wq_t = wq_d.ap().rearrange("(c p) f -> c p f", p=P)
    wk_t = wk_d.ap().rearrange("(c p) f -> c p f", p=P)
    wv_t = wv_d.ap().rearrange("(c p) f -> c p f", p=P)
    wo_t = wo_d.ap().rearrange("(c p) o -> c p o", p=P)
    out_t = out_d.ap().rearrange("(t p) o -> t p o", p=P)

    with tile.TileContext(nc) as tc:
      for rep in range(reps):
        with (
            tc.tile_pool(name=f"wpool{rep}", bufs=1) as wpool,
            tc.tile_pool(name=f"xpool{rep}", bufs=16) as xpool,
            tc.tile_pool(name=f"qkpool{rep}", bufs=1) as qkpool,
            tc.tile_pool(name=f"vpool{rep}", bufs=1) as vpool,
            tc.tile_pool(name=f"apool{rep}", bufs=6) as apool,
            tc.tile_pool(name=f"cpool{rep}", bufs=1) as cpool,
            tc.tile_pool(name=f"opool{rep}", bufs=3) as opool,
            tc.tile_pool(name=f"obpool{rep}", bufs=3) as obpool,
            tc.tile_pool(name=f"spool{rep}", bufs=2) as spool,
            tc.tile_pool(name=f"mmps{rep}", bufs=2, space="PSUM") as mmps,
            tc.tile_pool(name=f"pvps{rep}", bufs=3, space="PSUM") as pvps,
            tc.tile_pool(name=f"prps{rep}", bufs=1, space="PSUM") as prps,
        ):
            # ---- DMA emission (queue order sets arrival order) ----
            w_sb = {}
            x_sb = {}

            def load_w(nm, src):
                for c in range(KC):
                    t = wpool.tile([P, F], bf16, tag=f"{nm}{c}",
                                   name=f"{nm}{c}")
                    nc.sync.dma_start(t[:], src[c])
                    w_sb[nm, c] = t

            def load_x(nm, src):
                for c in range(KC):
                    t = xpool.tile([P, S], bf16, tag="xt", name=f"x_{nm}{c}")
                    nc.sync.dma_start(t[:], src[c])
                    x_sb[nm, c] = t

            # wk+xk first (kt prefix), then wv+xv (V projs), then
            # wq+xq (qt projs), wo last (output proj is emitted last).
            load_w("wk", wk_t)
            load_x("xk", xk_t)
            load_w("wv", wv_t)
            load_x("xv", xv_t)
            load_w("wq", wq_t)
            load_x("xq", xq_t)
            wo_sb = []
            for c in range(FC):
                t = wpool.tile([P, D], bf16, tag=f"wo{c}", name=f"wo{c}")
                nc.sync.dma_start(t[:], wo_t[c])
                wo_sb.append(t)

            # ---- projection helpers (emitted lazily) ----
            kts = [[None] * SG for _ in range(FC)]
            qts = [[None] * SG for _ in range(FC)]
            vts = [[None] * FC for _ in range(ST)]
            cxs = [[None] * SG for _ in range(FC)]

            def proj_kt(fc, sg):
                ps = prps.tile([P, 512], f32, tag="pr", name="ps_kt")
                for c in range(KC):
                    nc.tensor.matmul(
                        ps[:],
                        w_sb["wk", c][:, fc * P:(fc + 1) * P],
                        x_sb["xk", c][:, sg * 512:(sg + 1) * 512],
                        start=(c == 0), stop=(c == KC - 1),
                    )
                t = qkpool.tile([P, 512], bf16, tag=f"kt{fc}_{sg}",
                                name=f"kt{fc}_{sg}")
                nc.vector.tensor_copy(t[:], ps[:])
                kts[fc][sg] = t

            def proj_qt(fc, sg):
                ps = prps.tile([P, 512], f32, tag="pr", name="ps_qt")
                for c in range(KC):
                    nc.tensor.matmul(
                        ps[:],
                        w_sb["wq", c][:, fc * P:(fc + 1) * P],
                        x_sb["xq", c][:, sg * 512:(sg + 1) * 512],
                        start=(c == 0), stop=(c == KC - 1),
                    )
                t = qkpool.tile([P, 512], bf16, tag=f"qt{fc}_{sg}",
                                name=f"qt{fc}_{sg}")
                nc.vector.tensor_copy(t[:], ps[:])
                qts[fc][sg] = t

            def proj_v_full():
                # V projection, all heads: per seq tile t one [128, 512] MM
                # group, stored [128, 8, DK+1] with a per-head ones col.
                for t in range(ST):
                    ps = prps.tile([P, 512], f32, tag="pr", name="ps_v")
                    for c in range(KC):
                        nc.tensor.matmul(
                            ps[:],
                            x_sb["xv", c][:, t * P:(t + 1) * P],
                            w_sb["wv", c][:],
                            start=(c == 0), stop=(c == KC - 1),
                        )
                    vt = vpool.tile([P, HPC, DK + 1], bf16, tag=f"v{t}",
                                    name=f"v{t}")
                    nc.gpsimd.memset(vt[:, :, DK:DK + 1], 1.0)
                    nc.vector.tensor_copy(
                        vt[:, :, 0:DK],
                        ps.rearrange("p (h d) -> p h d", h=HPC))
                    for hp in range(FC):
                        vts[t][hp] = vt

            def attention(hp, sg):
                cps = [pvps.tile([DK + 1, 512], f32, tag="pv",
                                 name="ps_ctx") for _ in range(2)]
                for sk in range(ST):
                    sgk, off = divmod(sk, SG)
                    ps = mmps.tile([P, 1024], f32, tag="mm", name="ps_qk")
                    for h2 in range(2):
                        hq = slice(h2 * DK, (h2 + 1) * DK)
                        nc.tensor.matmul(
                            ps[:, h2 * 512:(h2 + 1) * 512],
                            kts[hp][sgk][hq, off * P:(off + 1) * P],
                            qts[hp][sg][hq, :],
                            start=True, stop=True,
                        )
                    at = apool.tile([P, 1024], bf16, tag="attn", name="attn")
                    nc.scalar.activation(at[:], ps[:], EXP, scale=0.125)
                    for h2 in range(2):
                        nc.tensor.matmul(
                            cps[h2][:],
                            vts[sk][hp][:, 2 * hp + h2, :],
                            at[:, h2 * 512:(h2 + 1) * 512],
                            start=(sk == 0), stop=(sk == ST - 1),
                        )
                # normalization: sums sit in row DK of each cps bank.
                ct = cpool.tile([P, 512], bf16, tag=f"cx{hp}_{sg}",
                                name=f"cx{hp}_{sg}")
                for h2 in range(2):
                    rin = spool.tile([1, 512], f32, tag="rin", name="rin")
                    nc.vector.reciprocal(rin[0:1, :], cps[h2][DK:DK + 1, :])
                    bcs = opool.tile([DK, 512], f32, tag="bcs", name="bcs")
                    nc.gpsimd.partition_broadcast(bcs[:], rin[0:1, :])
                    if h2 == 0:
                        nc.vector.tensor_mul(ct[0:DK, :], cps[h2][0:DK, :],
                                             bcs[:])
                    else:
                        tmp = opool.tile([DK, 512], bf16, tag="ctmp",
                                         name="ctmp")
                        nc.vector.tensor_mul(tmp[:], cps[h2][0:DK, :],
                                             bcs[:])
                        nc.sync.dma_start(ct[DK:P, :], tmp[:])
                cxs[hp][sg] = ct

            def outproj(sg):
                for tt in range(4):
                    t = sg * 4 + tt
                    for og in range(2):
                        ps = prps.tile([P, 512], f32, tag="pr",
                                       name="ps_out")
                        for fc in range(FC):
                            nc.tensor.matmul(
                                ps[:],
                                cxs[fc][sg][:, tt * P:(tt + 1) * P],
                                wo_sb[fc][:, og * 512:(og + 1) * 512],
                                start=(fc == 0), stop=(fc == FC - 1),
                            )
                        ot = obpool.tile([P, 512], f32, tag="out",
                                        name="out_sb")
                        nc.vector.tensor_copy(ot[:], ps[:])
                        nc.sync.dma_start(
                            out_t[t][:, og * 512:(og + 1) * 512], ot[:])

            # ---- emission order (sets scheduler priorities) ----
            # prefix: kt[0] first (DMA-paced), kt[1..3] as PE filler while
            # xv/xq stream in, then V(hp=0) and qt[0][0].
            for sg in range(SG):
                proj_kt(0, sg)
            proj_v_full()
            for fc in range(1, FC):
                for sg in range(SG):
                    proj_kt(fc, sg)
            proj_qt(0, 0)

            for sg in range(SG):
                for hp in range(FC):
                    if qts[hp][sg] is None:
                        proj_qt(hp, sg)
                    attention(hp, sg)
            for sg in range(SG):
                outproj(sg)

    nc.compile()
    return nc


_NC_CACHE = None


def _get_program():
    global _NC_CACHE
    if _NC_CACHE is None:
        _NC_CACHE = _build_program()
    return _NC_CACHE


def kernel(q, k, v, W_q, W_k, W_v, W_o):
    from concourse.bass_utils import run_bass_kernel_spmd

    q = np.asarray(q, np.float32)
    k = np.asarray(k, np.float32)
    v = np.asarray(v, np.float32)
    W_q = np.asarray(W_q, np.float32)
    W_k = np.asarray(W_k, np.float32)
    W_v = np.asarray(W_v, np.float32)
    W_o = np.asarray(W_o, np.float32)

    nc = _get_program()
    in_maps = []
    for c in range(NCORES):
        b, g = c // 2, c % 2
        sl = slice(g * F, (g + 1) * F)
        in_maps.append({
            "xq": np.ascontiguousarray(q[b].T).astype(BF),
            "xk": np.ascontiguousarray(k[b].T).astype(BF),
            "xv": np.ascontiguousarray(v[b].T).astype(BF),
            "wq": np.ascontiguousarray(W_q[sl, :].T).astype(BF),
            "wk": np.ascontiguousarray(W_k[sl, :].T).astype(BF),
            "wv": np.ascontiguousarray(W_v[sl, :].T).astype(BF),
            "wo": np.ascontiguousarray(W_o[:, sl].T).astype(BF),
        })
    res = run_bass_kernel_spmd(nc, in_maps, list(range(NCORES)))
    outs = [res.results[c]["out"] for c in range(NCORES)]
    full = np.stack([outs[2 * b] + outs[2 * b + 1] for b in range(4)])
    return full.astype(np.float32)


# revision 6
# speedup vs baseline: 1.6362x; 1.6362x over previous
"""Multi-head attention (B=4, S=2048, D=1024, H=16, d_k=64) on 8 NeuronCores.

Sharding: core c handles batch b=c//2 and head-group g=c%2 (8 heads, 512
features). Each core computes Q/K/V projections for its head group,
attention, and a partial output projection (row-split W_o). Host sums the
two partial outputs per batch.

v2: software-pipelined emission. The attention phase is ACT(exp)-bound, so
all projection and output-projection matmuls that are not needed up front
are emitted to fill the Tensor-engine gaps inside the attention stream:
- prefix: kt[0] (DMA-paced), kt[1..3] as filler, V for head-pair 0,
  qt[0][sg=0] -> attention starts ~30us in.
- steady state: per (sg, hp) attention block; qt/V projections for later
  blocks and the per-sg output projection interleave into PE gaps.
- per-(fc,sg) split tiles for qt/kt/ctx and per-(t,hp) V tiles keep
  producer/consumer dependencies tile-granular (no false serialization).

Numerics identical to v1 (all matmuls bf16, fp32 PSUM, exp without
max-subtraction is safe: |score/8| <= ~7).
"""

import sys

sys.path.insert(0, "/opt/trn_rl_repo")

import numpy as np
import ml_dtypes

BF = ml_dtypes.bfloat16

S = 2048          # sequence length
D = 1024          # model dim
F = 512           # features per core (8 heads x 64)
HPC = 8           # heads per core
DK = 64           # head dim
P = 128           # partitions
NCORES = 8
KC = D // P       # 8 contraction chunks for projections
ST = S // P       # 16 sequence tiles of 128
SG = S // 512     # 4 sequence groups of 512
FC = F // P       # 4 feature chunks of 128 (= head pairs)


def _build_program(reps=1):
    import concourse.bass as bass
    import concourse.mybir as mybir
    import concourse.tile as tile
    from concourse import bacc

    dt = mybir.dt
    f32 = dt.float32
    bf16 = dt.bfloat16
    EXP = mybir.ActivationFunctionType.Exp

    nc = bacc.Bacc("TRN2", target_bir_lowering=False, debug=False,
                   num_devices=NCORES)

    xq_d = nc.declare_dram_parameter("xq", [D, S], bf16, isOutput=False)
    xk_d = nc.declare_dram_parameter("xk", [D, S], bf16, isOutput=False)
    xv_d = nc.declare_dram_parameter("xv", [D, S], bf16, isOutput=False)
    wq_d = nc.declare_dram_parameter("wq", [D, F], bf16, isOutput=False)
    wk_d = nc.declare_dram_parameter("wk", [D, F], bf16, isOutput=False)
    wv_d = nc.declare_dram_parameter("wv", [D, F], bf16, isOutput=False)
    wo_d = nc.declare_dram_parameter("wo", [F, D], bf16, isOutput=False)
    out_d = nc.declare_dram_parameter("out", [S, D], f32, isOutput=True)

    xq_t = xq_d.ap().rearrange("(c p) s -> c p s", p=P)
    xk_t = xk_d.ap().rearrange("(c p) s -> c p s", p=P)
    xv_t = xv_d.ap().rearrange("(c p) s -> c p s", p=P)
    wq_t = wq_d.ap().rearrange("(c p) f -> c p f", p=P)
    wk_t = wk_d.ap().rearrange("(c p) f -> c p f", p=P)
    wv_t = wv_d.ap().rearrange("(c p) f -> c p f", p=P)
    wo_t = wo_d.ap().rearrange("(c p) o -> c p o", p=P)
    out_t = out_d.ap().rearrange("(t p) o -> t p o", p=P)

    with tile.TileContext(nc) as tc:
      for rep in range(reps):
        with (
            tc.tile_pool(name=f"wpool{rep}", bufs=1) as wpool,
            tc.tile_pool(name=f"xpool{rep}", bufs=16) as xpool,
            tc.tile_pool(name=f"qkpool{rep}", bufs=1) as qkpool,
            tc.tile_pool(name=f"vpool{rep}", bufs=1) as vpool,
            tc.tile_pool(name=f"apool{rep}", bufs=6) as apool,
            tc.tile_pool(name=f"cpool{rep}", bufs=1) as cpool,
            tc.tile_pool(name=f"opool{rep}", bufs=3) as opool,
            tc.tile_pool(name=f"obpool{rep}", bufs=3) as obpool,
            tc.tile_pool(name=f"spool{rep}", bufs=2) as spool,
            tc.tile_pool(name=f"mmps{rep}", bufs=2, space="PSUM") as mmps,
            tc.tile_pool(name=f"pvps{rep}", bufs=2, space="PSUM") as pvps,
            tc.tile_pool(name=f"prps{rep}", bufs=2, space="PSUM") as prps,
        ):
            # ---- DMA emission (queue order sets arrival order) ----
            w_sb = {}
            x_sb = {}

            def load_w(nm, src):
                for c in range(KC):
                    t = wpool.tile([P, F], bf16, tag=f"{nm}{c}",
                                   name=f"{nm}{c}")
                    nc.sync.dma_start(t[:], src[c])
                    w_sb[nm, c] = t

            def load_x(nm, src):
                for c in range(KC):
                    t = xpool.tile([P, S], bf16, tag="xt", name=f"x_{nm}{c}")
                    nc.sync.dma_start(t[:], src[c])
                    x_sb[nm, c] = t

            # wk+xk first (kt prefix), then wv+xv (V projs), then
            # wq+xq (qt projs), wo last (output proj is emitted last).
            load_w("wk", wk_t)
            load_x("xk", xk_t)
            load_w("wv", wv_t)
            load_x("xv", xv_t)
            load_w("wq", wq_t)
            load_x("xq", xq_t)
            wo_sb = []
            for c in range(FC):
                t = wpool.tile([P, D], bf16, tag=f"wo{c}", name=f"wo{c}")
                nc.sync.dma_start(t[:], wo_t[c])
                wo_sb.append(t)

            # ---- projection helpers (emitted lazily) ----
            kts = [[None] * SG for _ in range(FC)]
            qts = [[None] * SG for _ in range(FC)]
            vts = [[None] * FC for _ in range(ST)]
            cxs = [[None] * SG for _ in range(FC)]

            def proj_kt(fc, sg):
                ps = prps.tile([P, 512], f32, tag="pr", name="ps_kt")
                for c in range(KC):
                    nc.tensor.matmul(
                        ps[:],
                        w_sb["wk", c][:, fc * P:(fc + 1) * P],
                        x_sb["xk", c][:, sg * 512:(sg + 1) * 512],
                        start=(c == 0), stop=(c == KC - 1),
                    )
                t = qkpool.tile([P, 512], bf16, tag=f"kt{fc}_{sg}",
                                name=f"kt{fc}_{sg}")
                nc.vector.tensor_copy(t[:], ps[:])
                kts[fc][sg] = t

            def proj_qt(fc, sg):
                ps = prps.tile([P, 512], f32, tag="pr", name="ps_qt")
                for c in range(KC):
                    nc.tensor.matmul(
                        ps[:],
                        w_sb["wq", c][:, fc * P:(fc + 1) * P],
                        x_sb["xq", c][:, sg * 512:(sg + 1) * 512],
                        start=(c == 0), stop=(c == KC - 1),
                    )
                t = qkpool.tile([P, 512], bf16, tag=f"qt{fc}_{sg}",
                                name=f"qt{fc}_{sg}")
                nc.vector.tensor_copy(t[:], ps[:])
                qts[fc][sg] = t

            def proj_v_full():
                # V projection, all heads: per seq tile t one [128, 512] MM
                # group, stored [128, 8, DK+1] with a per-head ones col.
                for t in range(ST):
                    ps = prps.tile([P, 512], f32, tag="pr", name="ps_v")
                    for c in range(KC):
                        nc.tensor.matmul(
                            ps[:],
                            x_sb["xv", c][:, t * P:(t + 1) * P],
                            w_sb["wv", c][:],
                            start=(c == 0), stop=(c == KC - 1),
                        )
                    vt = vpool.tile([P, HPC, DK + 1], bf16, tag=f"v{t}",
                                    name=f"v{t}")
                    nc.gpsimd.memset(vt[:, :, DK:DK + 1], 1.0)
                    nc.vector.tensor_copy(
                        vt[:, :, 0:DK],
                        ps.rearrange("p (h d) -> p h d", h=HPC))
                    for hp in range(FC):
                        vts[t][hp] = vt

            def attention(hp, sg):
                cps = [pvps.tile([DK + 1, 512], f32, tag="pv",
                                 name="ps_ctx") for _ in range(2)]
                for sk in range(ST):
                    sgk, off = divmod(sk, SG)
                    ps = mmps.tile([P, 1024], f32, tag="mm", name="ps_qk")
                    for h2 in range(2):
                        hq = slice(h2 * DK, (h2 + 1) * DK)
                        nc.tensor.matmul(
                            ps[:, h2 * 512:(h2 + 1) * 512],
                            kts[hp][sgk][hq, off * P:(off + 1) * P],
                            qts[hp][sg][hq, :],
                            start=True, stop=True,
                        )
                    at = apool.tile([P, 1024], bf16, tag="attn", name="attn")
                    nc.scalar.activation(at[:], ps[:], EXP, scale=0.125)
                    for h2 in range(2):
                        nc.tensor.matmul(
                            cps[h2][:],
                            vts[sk][hp][:, 2 * hp + h2, :],
                            at[:, h2 * 512:(h2 + 1) * 512],
                            start=(sk == 0), stop=(sk == ST - 1),
                        )
                # normalization: sums sit in row DK of each cps bank.
                # Copy PSUM->SBUF first so the PV accumulator banks free
                # quickly (pvps bufs=2 -> next block's PV reuses them).
                ctf = [spool.tile([DK + 1, 512], f32, tag=f"ctf{h2}",
                                  name="ctf") for h2 in range(2)]
                for h2 in range(2):
                    nc.vector.tensor_copy(ctf[h2][:], cps[h2][:])
                ct = cpool.tile([P, 512], bf16, tag=f"cx{hp}_{sg}",
                                name=f"cx{hp}_{sg}")
                for h2 in range(2):
                    rin = spool.tile([1, 512], f32, tag="rin", name="rin")
                    nc.vector.reciprocal(rin[0:1, :], ctf[h2][DK:DK + 1, :])
                    bcs = opool.tile([DK, 512], f32, tag="bcs", name="bcs")
                    nc.gpsimd.partition_broadcast(bcs[:], rin[0:1, :])
                    if h2 == 0:
                        nc.vector.tensor_mul(ct[0:DK, :], ctf[h2][0:DK, :],
                                             bcs[:])
                    else:
                        tmp = opool.tile([DK, 512], bf16, tag="ctmp",
                                         name="ctmp")
                        nc.vector.tensor_mul(tmp[:], ctf[h2][0:DK, :],
                                             bcs[:])
                        nc.sync.dma_start(ct[DK:P, :], tmp[:])
                cxs[hp][sg] = ct

            def outproj(sg):
                for tt in range(4):
                    t = sg * 4 + tt
                    for og in range(2):
                        ps = prps.tile([P, 512], f32, tag="pr",
                                       name="ps_out")
                        for fc in range(FC):
                            nc.tensor.matmul(
                                ps[:],
                                cxs[fc][sg][:, tt * P:(tt + 1) * P],
                                wo_sb[fc][:, og * 512:(og + 1) * 512],
                                start=(fc == 0), stop=(fc == FC - 1),
                            )
                        ot = obpool.tile([P, 512], f32, tag="out",
                                        name="out_sb")
                        nc.vector.tensor_copy(ot[:], ps[:])
                        nc.sync.dma_start(
                            out_t[t][:, og * 512:(og + 1) * 512], ot[:])

            # ---- emission order (sets scheduler priorities) ----
            # prefix: kt[0] first (DMA-paced), kt[1..3] as PE filler while
            # xv/xq stream in, then V(hp=0) and qt[0][0].
            for sg in range(SG):
                proj_kt(0, sg)
            proj_v_full()
            for fc in range(1, FC):
                for sg in range(SG):
                    proj_kt(fc, sg)
            proj_qt(0, 0)

            for sg in range(SG):
                for hp in range(FC):
                    if qts[hp][sg] is None:
                        proj_qt(hp, sg)
                    attention(hp, sg)
            for sg in range(SG):
                outproj(sg)

    nc.compile()
    return nc


_NC_CACHE = None


def _get_program():
    global _NC_CACHE
    if _NC_CACHE is None:
        _NC_CACHE = _build_program()
    return _NC_CACHE


def kernel(q, k, v, W_q, W_k, W_v, W_o):
    from concourse.bass_utils import run_bass_kernel_spmd

    q = np.asarray(q, np.float32)
    k = np.asarray(k, np.float32)
    v = np.asarray(v, np.float32)
    W_q = np.asarray(W_q, np.float32)
    W_k = np.asarray(W_k, np.float32)
    W_v = np.asarray(W_v, np.float32)
    W_o = np.asarray(W_o, np.float32)

    nc = _get_program()
    in_maps = []
    for c in range(NCORES):
        b, g = c // 2, c % 2
        sl = slice(g * F, (g + 1) * F)
        in_maps.append({
            "xq": np.ascontiguousarray(q[b].T).astype(BF),
            "xk": np.ascontiguousarray(k[b].T).astype(BF),
            "xv": np.ascontiguousarray(v[b].T).astype(BF),
            "wq": np.ascontiguousarray(W_q[sl, :].T).astype(BF),
            "wk": np.ascontiguousarray(W_k[sl, :].T).astype(BF),
            "wv": np.ascontiguousarray(W_v[sl, :].T).astype(BF),
            "wo": np.ascontiguousarray(W_o[:, sl].T).astype(BF),
        })
    res = run_bass_kernel_spmd(nc, in_maps, list(range(NCORES)))
    outs = [res.results[c]["out"] for c in range(NCORES)]
    full = np.stack([outs[2 * b] + outs[2 * b + 1] for b in range(4)])
    return full.astype(np.float32)
